# revision 5
# baseline (speedup 1.0000x reference)
"""Sparse attention mixer (B=2,S=2048,D=1024,H=16,window=256 causal-banded)
on 8 trn2 NeuronCores.

Sharding: data-parallel over batch (2) x tensor-parallel over head groups (4).
Core c handles batch c//4, heads [4*(c%4), 4*(c%4)+4). Each core computes its
qkv projection slice, banded attention for its 4 heads, and a partial
out-projection over its 256 local dims; the host sums the 4 partials per batch
and adds the output bias.

Mask structure: mask[i,j] = 0 if j <= i+256 else -1e9  (causal OR |i-j|<=256,
clamped). Per 128-row query block qi, key blocks 0..qi+1 are fully allowed,
block qi+2 is lower-triangular (a<=b in transposed [sk,sq] layout), blocks
>qi+2 fully masked (skipped).
"""

import sys
import types

import numpy as np

B, S, D, H = 2, 2048, 1024, 16
HD = 64          # head dim
HPC = 4          # heads per core
DL = HPC * HD    # 256 local dims per core
NCORES = 8
P = 128
NEG = np.float32(-1.0e9)
SCALE = float(HD) ** -0.5

# knobs for test harness
TRACE = False
TRACE_CORES = None
LAST_RESULTS = None

_MODULE_CACHE = {}


def _install_ntff_shim():
    """antenv.axon_hooks is absent in this image; register the NTFF profile
    hook via ctypes against the axon PJRT .so so trace=True works."""
    if 'antenv.axon_hooks' in sys.modules:
        return
    hook = None
    try:
        from trn_agent_boot.trn_boot import _ntff_profile_via_ctypes
        hook = _ntff_profile_via_ctypes('/opt/axon/libaxon_pjrt.so')
    except Exception:
        hook = None
    m = types.ModuleType('antenv.axon_hooks')
    m.get_axon_ntff_profile_hook = lambda: hook
    m.set_axon_ntff_profile_hook = lambda h: None
    sys.modules['antenv.axon_hooks'] = m


def _build_module():
    import concourse.mybir as mybir
    import concourse.tile as tile
    from concourse import bacc
    from concourse.bass import ts

    dt = mybir.dt
    f32 = dt.float32
    f32r = dt.float32r
    AF = mybir.ActivationFunctionType

    NSC = S // 512   # 4 s-chunks of 512
    ND = D // P      # 8 d-chunks
    NB = S // P      # 16 s-blocks of 128

    nc = bacc.Bacc('TRN2', target_bir_lowering=False, debug=False,
                   num_devices=NCORES)

    xT = nc.dram_tensor('xT', [D, S], f32r, kind='ExternalInput').ap()
    wqT = nc.dram_tensor('wqT', [D, DL], f32r, kind='ExternalInput').ap()
    wkT = nc.dram_tensor('wkT', [D, DL], f32r, kind='ExternalInput').ap()
    wvT = nc.dram_tensor('wvT', [D, DL], f32r, kind='ExternalInput').ap()
    woT = nc.dram_tensor('woT', [DL, D], f32r, kind='ExternalInput').ap()
    bq2 = nc.dram_tensor('bq2', [P, 2], f32, kind='ExternalInput').ap()
    bk2 = nc.dram_tensor('bk2', [P, 2], f32, kind='ExternalInput').ap()
    bvrow = nc.dram_tensor('bvrow', [1, DL], f32r, kind='ExternalInput').ap()
    maskT = nc.dram_tensor('maskT', [P, P], f32, kind='ExternalInput').ap()
    onesrow = nc.dram_tensor('onesrow', [1, P], f32r, kind='ExternalInput').ap()
    onescol = nc.dram_tensor('onescol', [P, 16, 1], f32r,
                             kind='ExternalInput').ap()
    out = nc.dram_tensor('out', [S, D], f32, kind='ExternalOutput').ap()

    def r(ap):
        return ap

    with tile.TileContext(nc) as tc:
        with (
            tc.tile_pool(name='const', bufs=1) as cpool,
            tc.tile_pool(name='wp', bufs=1) as wpool,
            tc.tile_pool(name='xs', bufs=2) as xpool,
            tc.tile_pool(name='persist', bufs=1) as ppool,
            tc.tile_pool(name='expp', bufs=3) as epool,
            tc.tile_pool(name='rp', bufs=2) as rpool,
            tc.tile_pool(name='ostage', bufs=3) as opool,
            tc.tile_pool(name='mm', bufs=3, space='PSUM') as mmp,
            tc.tile_pool(name='vps', bufs=2, space='PSUM') as vpsp,
            tc.tile_pool(name='avo', bufs=2, space='PSUM') as avop,
        ):
            # ---------------- constants & weights ----------------
            mask_sb = cpool.tile([P, P], f32, name='mask_sb')
            nc.sync.dma_start(mask_sb[:], maskT)
            ones_sb = cpool.tile([1, P], f32r, name='ones_sb')
            nc.sync.dma_start(ones_sb[:], onesrow)
            bq_sb = cpool.tile([P, 2], f32, name='bq_sb')
            nc.sync.dma_start(bq_sb[:], bq2)
            bk_sb = cpool.tile([P, 2], f32, name='bk_sb')
            nc.sync.dma_start(bk_sb[:], bk2)
            bv_sb = cpool.tile([1, DL], f32r, name='bv_sb')
            nc.sync.dma_start(bv_sb[:], bvrow)

            wq_sb = wpool.tile([P, ND, DL], f32r, name='wq_sb')
            nc.sync.dma_start(wq_sb[:], wqT.rearrange('(c p) o -> p c o', p=P))
            wk_sb = wpool.tile([P, ND, DL], f32r, name='wk_sb')
            nc.sync.dma_start(wk_sb[:], wkT.rearrange('(c p) o -> p c o', p=P))
            wv_sb = wpool.tile([P, ND, DL], f32r, name='wv_sb')
            nc.sync.dma_start(wv_sb[:], wvT.rearrange('(c p) o -> p c o', p=P))
            wo_sb = wpool.tile([P, 2, D], f32r, name='wo_sb')
            nc.sync.dma_start(wo_sb[:], woT.rearrange('(t p) o -> p t o', p=P))

            # ---------------- persistent intermediates ----------------
            # pair t holds heads {2t, 2t+1} stacked along partitions (64 each)
            qT_sb = [ppool.tile([P, S], f32r, name=f'qT{t}') for t in range(2)]
            kT_sb = [ppool.tile([P, S], f32r, name=f'kT{t}') for t in range(2)]
            # V blocks: per s-block, per head: 64 V columns + 1 ones column
            v_sb = ppool.tile([P, NB, HPC * (HD + 1)], f32r, name='v_sb')
            # attn outT pairs: partitions = 128 local dims of pair t, free = s
            aoT_sb = [ppool.tile([P, S], f32r, name=f'aoT{t}') for t in range(2)]

            # per-head ones columns of v_sb (DMA; memset cannot write f32r)
            for h in range(HPC):
                c0 = h * (HD + 1) + HD
                nc.sync.dma_start(v_sb[:, :, c0:c0 + 1], onescol)

            # ---------------- qkv projection ----------------
            for sc in range(NSC):
                xt = xpool.tile([P, ND, 512], f32r, name=f'xt{sc}', tag='xt')
                nc.sync.dma_start(
                    xt[:], xT.rearrange('(c p) s -> p c s', p=P)[:, :, ts(sc, 512)])
                # qT / kT pairs: out [128(o), 512(s)]
                for wsb, bsb, dst, scale in ((wq_sb, bq_sb, qT_sb, SCALE),
                                             (wk_sb, bk_sb, kT_sb, 1.0)):
                    for t in range(2):
                        ps = mmp.tile([P, 512], f32, name=f'qk_ps{sc}_{t}',
                                      tag='mm')
                        for c in range(ND):
                            nc.tensor.matmul(
                                ps[:], r(wsb[:, c, ts(t, P)]), r(xt[:, c, :]),
                                start=(c == 0), stop=(c == ND - 1))
                        nc.scalar.activation(dst[t][:, ts(sc, 512)], ps[:],
                                             AF.Identity,
                                             bias=bsb[:, t:t + 1], scale=scale)
                # V: out [128(s), 256(o)] per s-block, then scatter per head
                for sbl in range(4):
                    sb = 4 * sc + sbl
                    vps = vpsp.tile([P, DL], f32, name=f'v_ps{sb}', tag='vps')
                    for c in range(ND):
                        nc.tensor.matmul(
                            vps[:], r(xt[:, c, ts(sbl, P)]), r(wv_sb[:, c, :]),
                            start=(c == 0), stop=False)
                    nc.tensor.matmul(vps[:], r(ones_sb[0:1, :]),
                                     r(bv_sb[0:1, :]), start=False, stop=True)
                    for h in range(HPC):
                        nc.vector.tensor_copy(
                            v_sb[:, sb, h * (HD + 1):h * (HD + 1) + HD],
                            vps[:, ts(h, HD)])

            # ---------------- attention + out_proj ----------------
            for c in range(NSC):        # query chunk: s columns [512c, 512c+512)
                for h in range(HPC):
                    t, hi = h // 2, h % 2
                    kb_max = min(NB, 4 * c + 6)   # key blocks 0..kb_max-1
                    avo = avop.tile([HD + 1, 512], f32, name=f'avo{c}_{h}',
                                    tag='avo')
                    for kb in range(kb_max):
                        z = max(0, kb - 4 * c - 2)   # fully-masked sub-blocks
                        n0 = P * z
                        sps = mmp.tile([P, 512], f32, name=f's_ps{c}_{h}_{kb}',
                                       tag='mm')
                        nc.tensor.matmul(
                            sps[:, n0:],
                            r(kT_sb[t][64 * hi:64 * hi + 64, ts(kb, P)]),
                            r(qT_sb[t][64 * hi:64 * hi + 64,
                                       512 * c + n0:512 * (c + 1)]),
                            start=True, stop=True)
                        lb = kb - 2 - 4 * c          # banded sub-block index
                        if 0 <= lb < 4:
                            nc.vector.tensor_add(sps[:, ts(lb, P)],
                                                 sps[:, ts(lb, P)], mask_sb[:])
                        et = epool.tile([P, 512], f32r, name=f'exp{c}_{h}_{kb}',
                                        tag='exp')
                        nc.scalar.activation(et[:, n0:], sps[:, n0:], AF.Exp)
                        nc.tensor.matmul(
                            avo[:, n0:],
                            r(v_sb[:, kb, h * (HD + 1):(h + 1) * (HD + 1)]),
                            r(et[:, n0:]),
                            start=(kb == 0), stop=(kb == kb_max - 1),
                            skip_group_check=True)
                    # normalization: rows 0..63 = sum(attn*V), row 64 = sum(exp)
                    rsb = rpool.tile([1, 512], f32, name=f'r{c}_{h}', tag='r')
                    nc.vector.reciprocal(rsb[:], avo[64:65, :])
                    rb = rpool.tile([HD, 512], f32, name=f'rb{c}_{h}', tag='rb')
                    nc.gpsimd.partition_broadcast(rb[:], rsb[:])
                    nc.vector.tensor_mul(
                        aoT_sb[t][64 * hi:64 * hi + 64, ts(c, 512)],
                        avo[0:HD, :], rb[:])
                # out_proj for s-blocks 4c..4c+3 (need all heads of chunk c)
                for sbl in range(4):
                    m = 4 * c + sbl
                    for n in range(2):
                        ops = vpsp.tile([P, 512], f32, name=f'o_ps{m}_{n}',
                                        tag='vps')
                        for t in range(2):
                            nc.tensor.matmul(ops[:],
                                             r(aoT_sb[t][:, ts(m, P)]),
                                             r(wo_sb[:, t, ts(n, 512)]),
                                             start=(t == 0), stop=(t == 1))
                        ost = opool.tile([P, 512], f32, name=f'ost{m}_{n}',
                                         tag='ost')
                        nc.vector.tensor_copy(ost[:], ops[:])
                        nc.sync.dma_start(out[ts(m, P), ts(n, 512)], ost[:])

    nc.compile()
    return nc


def _get_module():
    if 'nc' not in _MODULE_CACHE:
        _MODULE_CACHE['nc'] = _build_module()
    return _MODULE_CACHE['nc']


def _make_in_maps(x, in_proj_w, in_proj_b, out_proj_w):
    x = np.asarray(x, np.float32)
    in_proj_w = np.asarray(in_proj_w, np.float32)
    in_proj_b = np.asarray(in_proj_b, np.float32)
    out_proj_w = np.asarray(out_proj_w, np.float32)

    mask = np.where(np.arange(P)[:, None] <= np.arange(P)[None, :],
                    np.float32(0.0), NEG).astype(np.float32)
    ones = np.ones((1, P), np.float32)

    xT = [np.ascontiguousarray(x[b].T) for b in range(B)]
    in_maps = []
    for core in range(NCORES):
        b, hg = core // 4, core % 4
        sl = slice(DL * hg, DL * hg + DL)
        wq = in_proj_w[0 * D:1 * D][sl]
        wk = in_proj_w[1 * D:2 * D][sl]
        wv = in_proj_w[2 * D:3 * D][sl]
        bq = in_proj_b[0 * D:1 * D][sl]
        bk = in_proj_b[1 * D:2 * D][sl]
        bv = in_proj_b[2 * D:3 * D][sl]
        in_maps.append({
            'xT': xT[b],
            'wqT': np.ascontiguousarray(wq.T),
            'wkT': np.ascontiguousarray(wk.T),
            'wvT': np.ascontiguousarray(wv.T),
            'woT': np.ascontiguousarray(out_proj_w[:, sl].T),
            'bq2': np.ascontiguousarray((bq * SCALE).reshape(2, P).T),
            'bk2': np.ascontiguousarray(bk.reshape(2, P).T),
            'bvrow': bv.reshape(1, DL).copy(),
            'maskT': mask,
            'onesrow': ones,
            'onescol': np.ones((P, 16, 1), np.float32),
        })
    return in_maps


def kernel(x, in_proj_w, in_proj_b, out_proj_w, out_proj_b):
    global LAST_RESULTS
    _install_ntff_shim()
    from concourse import bass_utils

    nc = _get_module()
    in_maps = _make_in_maps(x, in_proj_w, in_proj_b, out_proj_w)
    res = bass_utils.run_bass_kernel_spmd(
        nc, in_maps, core_ids=list(range(NCORES)),
        trace=TRACE,
        **({'trace_cores': TRACE_CORES} if TRACE_CORES else {}))
    LAST_RESULTS = res

    out = np.zeros((B, S, D), np.float32)
    for core in range(NCORES):
        out[core // 4] += res.results[core]['out']
    out += np.asarray(out_proj_b, np.float32)
    return out


# revision 10
# speedup vs baseline: 1.0096x; 1.0096x over previous
"""Sparse attention mixer (B=2,S=2048,D=1024,H=16,window=256 causal-banded)
on 8 trn2 NeuronCores.

Sharding: data-parallel over batch (2) x tensor-parallel over head groups (4).
Core c handles batch c//4, heads [4*(c%4), 4*(c%4)+4). Each core computes its
qkv projection slice, banded attention for its 4 heads, and a partial
out-projection over its 256 local dims; the host sums the 4 partials per batch
and adds the output bias.

Mask structure: mask[i,j] = 0 if j <= i+256 else -1e9  (causal OR |i-j|<=256,
clamped). Per 128-row query block qi, key blocks 0..qi+1 are fully allowed,
block qi+2 is lower-triangular (a<=b in transposed [sk,sq] layout), blocks
>qi+2 fully masked (skipped).
"""

import sys
import types

import numpy as np

B, S, D, H = 2, 2048, 1024, 16
HD = 64          # head dim
HPC = 4          # heads per core
DL = HPC * HD    # 256 local dims per core
NCORES = 8
P = 128
NEG = np.float32(-1.0e9)
SCALE = float(HD) ** -0.5

# knobs for test harness
TRACE = False
TRACE_CORES = None
LAST_RESULTS = None

_MODULE_CACHE = {}


def _install_ntff_shim():
    """antenv.axon_hooks is absent in this image; register the NTFF profile
    hook via ctypes against the axon PJRT .so so trace=True works."""
    if 'antenv.axon_hooks' in sys.modules:
        return
    hook = None
    try:
        from trn_agent_boot.trn_boot import _ntff_profile_via_ctypes
        hook = _ntff_profile_via_ctypes('/opt/axon/libaxon_pjrt.so')
    except Exception:
        hook = None
    m = types.ModuleType('antenv.axon_hooks')
    m.get_axon_ntff_profile_hook = lambda: hook
    m.set_axon_ntff_profile_hook = lambda h: None
    sys.modules['antenv.axon_hooks'] = m


def _build_module():
    import concourse.mybir as mybir
    import concourse.tile as tile
    from concourse import bacc
    from concourse.bass import ts

    dt = mybir.dt
    f32 = dt.float32
    f32r = dt.float32r
    AF = mybir.ActivationFunctionType

    NSC = S // 512   # 4 s-chunks of 512
    ND = D // P      # 8 d-chunks
    NB = S // P      # 16 s-blocks of 128

    nc = bacc.Bacc('TRN2', target_bir_lowering=False, debug=False,
                   num_devices=NCORES)

    xT = nc.dram_tensor('xT', [D, S], f32r, kind='ExternalInput').ap()
    wqT = nc.dram_tensor('wqT', [D, DL], f32r, kind='ExternalInput').ap()
    wkT = nc.dram_tensor('wkT', [D, DL], f32r, kind='ExternalInput').ap()
    wvT = nc.dram_tensor('wvT', [D, DL], f32r, kind='ExternalInput').ap()
    woT = nc.dram_tensor('woT', [DL, D], f32r, kind='ExternalInput').ap()
    bq2 = nc.dram_tensor('bq2', [P, 2], f32, kind='ExternalInput').ap()
    bk2 = nc.dram_tensor('bk2', [P, 2], f32, kind='ExternalInput').ap()
    bvrow = nc.dram_tensor('bvrow', [1, DL], f32, kind='ExternalInput').ap()
    maskT = nc.dram_tensor('maskT', [P, P], f32, kind='ExternalInput').ap()
    onescol = nc.dram_tensor('onescol', [P, 16, 1], f32r,
                             kind='ExternalInput').ap()
    out = nc.dram_tensor('out', [S, D], f32, kind='ExternalOutput').ap()

    def r(ap):
        return ap

    def act_reciprocal(out_ap, in_ap):
        # scalar.activation() refuses Reciprocal (accuracy ~1e-5, fine here);
        # emit InstActivation directly: ins = [in, bias, scale, alpha]
        eng = nc.scalar
        ins = [eng.lower_ap(in_ap),
               mybir.ImmediateValue(dtype=f32, value=0.0),
               mybir.ImmediateValue(dtype=f32, value=1.0),
               mybir.ImmediateValue(dtype=f32, value=0.0)]
        eng.add_instruction(mybir.InstActivation(
            name=nc.get_next_instruction_name(),
            func=AF.Reciprocal, ins=ins, outs=[eng.lower_ap(out_ap)]))

    with tile.TileContext(nc) as tc:
        with (
            tc.tile_pool(name='const', bufs=1) as cpool,
            tc.tile_pool(name='wp', bufs=1) as wpool,
            tc.tile_pool(name='xs', bufs=2) as xpool,
            tc.tile_pool(name='persist', bufs=1) as ppool,
            tc.tile_pool(name='expp', bufs=3) as epool,
            tc.tile_pool(name='rp', bufs=2) as rpool,
            tc.tile_pool(name='ostage', bufs=3) as opool,
            tc.tile_pool(name='mm', bufs=3, space='PSUM') as mmp,
            tc.tile_pool(name='vps', bufs=2, space='PSUM') as vpsp,
            tc.tile_pool(name='avo', bufs=3, space='PSUM') as avop,
        ):
            # ---------------- constants & weights ----------------
            mask_sb = cpool.tile([P, P], f32, name='mask_sb')
            nc.sync.dma_start(mask_sb[:], maskT)
            bq_sb = cpool.tile([P, 2], f32, name='bq_sb')
            nc.sync.dma_start(bq_sb[:], bq2)
            bk_sb = cpool.tile([P, 2], f32, name='bk_sb')
            nc.sync.dma_start(bk_sb[:], bk2)
            bv_sb = cpool.tile([1, DL], f32, name='bv_sb')
            nc.sync.dma_start(bv_sb[:], bvrow)
            # V bias broadcast to all partitions (added during V psum->sbuf)
            bvb_sb = cpool.tile([P, DL], f32, name='bvb_sb')
            nc.gpsimd.partition_broadcast(bvb_sb[:], bv_sb[:])

            wq_sb = wpool.tile([P, ND, DL], f32r, name='wq_sb')
            nc.sync.dma_start(wq_sb[:], wqT.rearrange('(c p) o -> p c o', p=P))
            wk_sb = wpool.tile([P, ND, DL], f32r, name='wk_sb')
            nc.sync.dma_start(wk_sb[:], wkT.rearrange('(c p) o -> p c o', p=P))
            wv_sb = wpool.tile([P, ND, DL], f32r, name='wv_sb')
            nc.sync.dma_start(wv_sb[:], wvT.rearrange('(c p) o -> p c o', p=P))
            wo_sb = wpool.tile([P, 2, D], f32r, name='wo_sb')
            nc.sync.dma_start(wo_sb[:], woT.rearrange('(t p) o -> p t o', p=P))

            # ---------------- persistent intermediates ----------------
            # pair t holds heads {2t, 2t+1} stacked along partitions (64 each)
            qT_sb = [ppool.tile([P, S], f32r, name=f'qT{t}') for t in range(2)]
            kT_sb = [ppool.tile([P, S], f32r, name=f'kT{t}') for t in range(2)]
            # V blocks: per s-block, per head: 64 V columns + 1 ones column
            v_sb = ppool.tile([P, NB, HPC * (HD + 1)], f32r, name='v_sb')
            # attn outT pairs: partitions = 128 local dims of pair t, free = s
            aoT_sb = [ppool.tile([P, S], f32r, name=f'aoT{t}') for t in range(2)]

            # per-head ones columns of v_sb (DMA; memset cannot write f32r)
            for h in range(HPC):
                c0 = h * (HD + 1) + HD
                nc.sync.dma_start(v_sb[:, :, c0:c0 + 1], onescol)

            # ---------------- qkv projection ----------------
            for sc in range(NSC):
                xt = xpool.tile([P, ND, 512], f32r, name=f'xt{sc}', tag='xt')
                nc.sync.dma_start(
                    xt[:], xT.rearrange('(c p) s -> p c s', p=P)[:, :, ts(sc, 512)])
                # qT / kT pairs: out [128(o), 512(s)]
                for wsb, bsb, dst, scale in ((wq_sb, bq_sb, qT_sb, SCALE),
                                             (wk_sb, bk_sb, kT_sb, 1.0)):
                    for t in range(2):
                        ps = mmp.tile([P, 512], f32, name=f'qk_ps{sc}_{t}',
                                      tag='mm')
                        for c in range(ND):
                            nc.tensor.matmul(
                                ps[:], r(wsb[:, c, ts(t, P)]), r(xt[:, c, :]),
                                start=(c == 0), stop=(c == ND - 1))
                        nc.scalar.activation(dst[t][:, ts(sc, 512)], ps[:],
                                             AF.Identity,
                                             bias=bsb[:, t:t + 1], scale=scale)
                # V: out [128(s), 256(o)] per s-block, then scatter per head
                # (bias added on the psum->sbuf move)
                for sbl in range(4):
                    sb = 4 * sc + sbl
                    vps = vpsp.tile([P, DL], f32, name=f'v_ps{sb}', tag='vps')
                    for c in range(ND):
                        nc.tensor.matmul(
                            vps[:], r(xt[:, c, ts(sbl, P)]), r(wv_sb[:, c, :]),
                            start=(c == 0), stop=(c == ND - 1))
                    for h in range(HPC):
                        nc.vector.tensor_add(
                            v_sb[:, sb, h * (HD + 1):h * (HD + 1) + HD],
                            vps[:, ts(h, HD)], bvb_sb[:, ts(h, HD)])

            # ---------------- attention + out_proj ----------------
            for c in range(NSC):        # query chunk: s columns [512c, 512c+512)
                for t in range(2):      # head pair; heads 2t (rows 0:64), 2t+1
                    kb_max = min(NB, 4 * c + 6)   # key blocks 0..kb_max-1
                    avo = [avop.tile([HD + 1, 512], f32,
                                     name=f'avo{c}_{2 * t + hi}', tag='avo')
                           for hi in range(2)]
                    for kb in range(kb_max):
                        z = max(0, kb - 4 * c - 2)   # fully-masked sub-blocks
                        n0 = P * z
                        lb = kb - 2 - 4 * c          # banded sub-block index
                        sps = [mmp.tile([P, 512], f32,
                                        name=f's_ps{c}_{t}_{kb}_{hi}', tag='mm')
                               for hi in range(2)]
                        # the two heads' K=64 matmuls sit in distinct PE row
                        # groups (partitions 0:64 / 64:128) and overlap
                        for hi in range(2):
                            nc.tensor.matmul(
                                sps[hi][:, n0:],
                                r(kT_sb[t][64 * hi:64 * hi + 64, ts(kb, P)]),
                                r(qT_sb[t][64 * hi:64 * hi + 64,
                                           512 * c + n0:512 * (c + 1)]),
                                start=True, stop=True)
                        for hi in range(2):
                            h = 2 * t + hi
                            if 0 <= lb < 4:
                                nc.vector.tensor_add(sps[hi][:, ts(lb, P)],
                                                     sps[hi][:, ts(lb, P)],
                                                     mask_sb[:])
                            et = epool.tile([P, 512], f32r,
                                            name=f'exp{c}_{h}_{kb}', tag='exp')
                            nc.scalar.activation(et[:, n0:], sps[hi][:, n0:],
                                                 AF.Exp)
                            nc.tensor.matmul(
                                avo[hi][:, n0:],
                                r(v_sb[:, kb, h * (HD + 1):(h + 1) * (HD + 1)]),
                                r(et[:, n0:]),
                                start=(kb == 0), stop=(kb == kb_max - 1),
                                skip_group_check=True)
                    # normalize: avo rows 0..63 = sum(attn*V), row 64 = sum(exp)
                    for hi in range(2):
                        h = 2 * t + hi
                        rsb = rpool.tile([1, 512], f32, name=f'r{c}_{h}',
                                         tag='r')
                        act_reciprocal(rsb[:], avo[hi][64:65, :])
                        rb = rpool.tile([HD, 512], f32, name=f'rb{c}_{h}',
                                        tag='rb')
                        nc.gpsimd.partition_broadcast(rb[:], rsb[:])
                        nc.vector.tensor_mul(
                            aoT_sb[t][64 * hi:64 * hi + 64, ts(c, 512)],
                            avo[hi][0:HD, :], rb[:])
                # out_proj for s-blocks 4c..4c+3 (need all heads of chunk c)
                for sbl in range(4):
                    m = 4 * c + sbl
                    for n in range(2):
                        ops = vpsp.tile([P, 512], f32, name=f'o_ps{m}_{n}',
                                        tag='vps')
                        for t in range(2):
                            nc.tensor.matmul(ops[:],
                                             r(aoT_sb[t][:, ts(m, P)]),
                                             r(wo_sb[:, t, ts(n, 512)]),
                                             start=(t == 0), stop=(t == 1))
                        ost = opool.tile([P, 512], f32, name=f'ost{m}_{n}',
                                         tag='ost')
                        nc.vector.tensor_copy(ost[:], ops[:])
                        nc.sync.dma_start(out[ts(m, P), ts(n, 512)], ost[:])

    nc.compile()
    return nc


def _get_module():
    if 'nc' not in _MODULE_CACHE:
        _MODULE_CACHE['nc'] = _build_module()
    return _MODULE_CACHE['nc']


def _make_in_maps(x, in_proj_w, in_proj_b, out_proj_w):
    x = np.asarray(x, np.float32)
    in_proj_w = np.asarray(in_proj_w, np.float32)
    in_proj_b = np.asarray(in_proj_b, np.float32)
    out_proj_w = np.asarray(out_proj_w, np.float32)

    mask = np.where(np.arange(P)[:, None] <= np.arange(P)[None, :],
                    np.float32(0.0), NEG).astype(np.float32)

    xT = [np.ascontiguousarray(x[b].T) for b in range(B)]
    in_maps = []
    for core in range(NCORES):
        b, hg = core // 4, core % 4
        sl = slice(DL * hg, DL * hg + DL)
        wq = in_proj_w[0 * D:1 * D][sl]
        wk = in_proj_w[1 * D:2 * D][sl]
        wv = in_proj_w[2 * D:3 * D][sl]
        bq = in_proj_b[0 * D:1 * D][sl]
        bk = in_proj_b[1 * D:2 * D][sl]
        bv = in_proj_b[2 * D:3 * D][sl]
        in_maps.append({
            'xT': xT[b],
            'wqT': np.ascontiguousarray(wq.T),
            'wkT': np.ascontiguousarray(wk.T),
            'wvT': np.ascontiguousarray(wv.T),
            'woT': np.ascontiguousarray(out_proj_w[:, sl].T),
            'bq2': np.ascontiguousarray((bq * SCALE).reshape(2, P).T),
            'bk2': np.ascontiguousarray(bk.reshape(2, P).T),
            'bvrow': bv.reshape(1, DL).copy(),
            'maskT': mask,
            'onescol': np.ones((P, 16, 1), np.float32),
        })
    return in_maps


def kernel(x, in_proj_w, in_proj_b, out_proj_w, out_proj_b):
    global LAST_RESULTS
    _install_ntff_shim()
    from concourse import bass_utils

    nc = _get_module()
    in_maps = _make_in_maps(x, in_proj_w, in_proj_b, out_proj_w)
    res = bass_utils.run_bass_kernel_spmd(
        nc, in_maps, core_ids=list(range(NCORES)),
        trace=TRACE,
        **({'trace_cores': TRACE_CORES} if TRACE_CORES else {}))
    LAST_RESULTS = res

    out = np.zeros((B, S, D), np.float32)
    for core in range(NCORES):
        out[core // 4] += res.results[core]['out']
    out += np.asarray(out_proj_b, np.float32)
    return out


# revision 11
# speedup vs baseline: 1.1908x; 1.1794x over previous
"""Sparse attention mixer (B=2,S=2048,D=1024,H=16,window=256 causal-banded)
on 8 trn2 NeuronCores.

Sharding: data-parallel over batch (2) x tensor-parallel over head groups (4).
Core c handles batch c//4, heads [4*(c%4), 4*(c%4)+4). Each core computes its
qkv projection slice, banded attention for its 4 heads, and a partial
out-projection over its 256 local dims; the host sums the 4 partials per batch
and adds the output bias.

Mask structure: mask[i,j] = 0 if j <= i+256 else -1e9  (causal OR |i-j|<=256,
clamped). Per 128-row query block qi, key blocks 0..qi+1 are fully allowed,
block qi+2 is lower-triangular (a<=b in transposed [sk,sq] layout), blocks
>qi+2 fully masked (skipped).
"""

import sys
import types

import numpy as np

B, S, D, H = 2, 2048, 1024, 16
HD = 64          # head dim
HPC = 4          # heads per core
DL = HPC * HD    # 256 local dims per core
NCORES = 8
P = 128
NEG = np.float32(-1.0e9)
SCALE = float(HD) ** -0.5

# knobs for test harness
TRACE = False
TRACE_CORES = None
LAST_RESULTS = None

_MODULE_CACHE = {}


def _install_ntff_shim():
    """antenv.axon_hooks is absent in this image; register the NTFF profile
    hook via ctypes against the axon PJRT .so so trace=True works."""
    if 'antenv.axon_hooks' in sys.modules:
        return
    hook = None
    try:
        from trn_agent_boot.trn_boot import _ntff_profile_via_ctypes
        hook = _ntff_profile_via_ctypes('/opt/axon/libaxon_pjrt.so')
    except Exception:
        hook = None
    m = types.ModuleType('antenv.axon_hooks')
    m.get_axon_ntff_profile_hook = lambda: hook
    m.set_axon_ntff_profile_hook = lambda h: None
    sys.modules['antenv.axon_hooks'] = m


def _build_module():
    import concourse.mybir as mybir
    import concourse.tile as tile
    from concourse import bacc
    from concourse.bass import ts

    dt = mybir.dt
    f32 = dt.float32
    f32r = dt.float32r
    bf16 = dt.bfloat16
    AF = mybir.ActivationFunctionType

    NSC = S // 512   # 4 s-chunks of 512
    ND = D // P      # 8 d-chunks
    NB = S // P      # 16 s-blocks of 128

    nc = bacc.Bacc('TRN2', target_bir_lowering=False, debug=False,
                   num_devices=NCORES)

    xT = nc.dram_tensor('xT', [D, S], bf16, kind='ExternalInput').ap()
    wqT = nc.dram_tensor('wqT', [D, DL], bf16, kind='ExternalInput').ap()
    wkT = nc.dram_tensor('wkT', [D, DL], bf16, kind='ExternalInput').ap()
    wvT = nc.dram_tensor('wvT', [D, DL], bf16, kind='ExternalInput').ap()
    woT = nc.dram_tensor('woT', [DL, D], bf16, kind='ExternalInput').ap()
    bq2 = nc.dram_tensor('bq2', [P, 2], f32, kind='ExternalInput').ap()
    bk2 = nc.dram_tensor('bk2', [P, 2], f32, kind='ExternalInput').ap()
    bvrow = nc.dram_tensor('bvrow', [1, DL], f32, kind='ExternalInput').ap()
    maskT = nc.dram_tensor('maskT', [P, P], f32, kind='ExternalInput').ap()
    onescol = nc.dram_tensor('onescol', [P, 16, 1], bf16,
                             kind='ExternalInput').ap()
    out = nc.dram_tensor('out', [S, D], f32, kind='ExternalOutput').ap()

    def r(ap):
        return ap

    def act_reciprocal(out_ap, in_ap):
        # scalar.activation() refuses Reciprocal (accuracy ~1e-5, fine here);
        # emit InstActivation directly: ins = [in, bias, scale, alpha]
        eng = nc.scalar
        ins = [eng.lower_ap(in_ap),
               mybir.ImmediateValue(dtype=f32, value=0.0),
               mybir.ImmediateValue(dtype=f32, value=1.0),
               mybir.ImmediateValue(dtype=f32, value=0.0)]
        eng.add_instruction(mybir.InstActivation(
            name=nc.get_next_instruction_name(),
            func=AF.Reciprocal, ins=ins, outs=[eng.lower_ap(out_ap)]))

    with tile.TileContext(nc) as tc:
        with (
            tc.tile_pool(name='const', bufs=1) as cpool,
            tc.tile_pool(name='wp', bufs=1) as wpool,
            tc.tile_pool(name='xs', bufs=2) as xpool,
            tc.tile_pool(name='persist', bufs=1) as ppool,
            tc.tile_pool(name='expp', bufs=3) as epool,
            tc.tile_pool(name='rp', bufs=2) as rpool,
            tc.tile_pool(name='ostage', bufs=3) as opool,
            tc.tile_pool(name='mm', bufs=3, space='PSUM') as mmp,
            tc.tile_pool(name='vps', bufs=2, space='PSUM') as vpsp,
            tc.tile_pool(name='avo', bufs=3, space='PSUM') as avop,
        ):
            # ---------------- constants & weights ----------------
            mask_sb = cpool.tile([P, P], f32, name='mask_sb')
            nc.sync.dma_start(mask_sb[:], maskT)
            bq_sb = cpool.tile([P, 2], f32, name='bq_sb')
            nc.sync.dma_start(bq_sb[:], bq2)
            bk_sb = cpool.tile([P, 2], f32, name='bk_sb')
            nc.sync.dma_start(bk_sb[:], bk2)
            bv_sb = cpool.tile([1, DL], f32, name='bv_sb')
            nc.sync.dma_start(bv_sb[:], bvrow)
            # V bias broadcast to all partitions (added during V psum->sbuf)
            bvb_sb = cpool.tile([P, DL], f32, name='bvb_sb')
            nc.gpsimd.partition_broadcast(bvb_sb[:], bv_sb[:])

            wq_sb = wpool.tile([P, ND, DL], bf16, name='wq_sb')
            nc.sync.dma_start(wq_sb[:], wqT.rearrange('(c p) o -> p c o', p=P))
            wk_sb = wpool.tile([P, ND, DL], bf16, name='wk_sb')
            nc.sync.dma_start(wk_sb[:], wkT.rearrange('(c p) o -> p c o', p=P))
            wv_sb = wpool.tile([P, ND, DL], bf16, name='wv_sb')
            nc.sync.dma_start(wv_sb[:], wvT.rearrange('(c p) o -> p c o', p=P))
            wo_sb = wpool.tile([P, 2, D], bf16, name='wo_sb')
            nc.sync.dma_start(wo_sb[:], woT.rearrange('(t p) o -> p t o', p=P))

            # ---------------- persistent intermediates ----------------
            # pair t holds heads {2t, 2t+1} stacked along partitions (64 each)
            qT_sb = [ppool.tile([P, S], bf16, name=f'qT{t}') for t in range(2)]
            kT_sb = [ppool.tile([P, S], bf16, name=f'kT{t}') for t in range(2)]
            # V blocks: per s-block, per head: 64 V columns + 1 ones column
            v_sb = ppool.tile([P, NB, HPC * (HD + 1)], bf16, name='v_sb')
            # attn outT pairs: partitions = 128 local dims of pair t, free = s
            aoT_sb = [ppool.tile([P, S], bf16, name=f'aoT{t}') for t in range(2)]

            # per-head ones columns of v_sb (DMA; memset cannot write f32r)
            for h in range(HPC):
                c0 = h * (HD + 1) + HD
                nc.sync.dma_start(v_sb[:, :, c0:c0 + 1], onescol)

            # ---------------- qkv projection ----------------
            for sc in range(NSC):
                xt = xpool.tile([P, ND, 512], bf16, name=f'xt{sc}', tag='xt')
                nc.sync.dma_start(
                    xt[:], xT.rearrange('(c p) s -> p c s', p=P)[:, :, ts(sc, 512)])
                # qT / kT pairs: out [128(o), 512(s)]
                for wsb, bsb, dst, scale in ((wq_sb, bq_sb, qT_sb, SCALE),
                                             (wk_sb, bk_sb, kT_sb, 1.0)):
                    for t in range(2):
                        ps = mmp.tile([P, 512], f32, name=f'qk_ps{sc}_{t}',
                                      tag='mm')
                        for c in range(ND):
                            nc.tensor.matmul(
                                ps[:], r(wsb[:, c, ts(t, P)]), r(xt[:, c, :]),
                                start=(c == 0), stop=(c == ND - 1))
                        nc.scalar.activation(dst[t][:, ts(sc, 512)], ps[:],
                                             AF.Identity,
                                             bias=bsb[:, t:t + 1], scale=scale)
                # V: out [128(s), 256(o)] per s-block, then scatter per head
                # (bias added on the psum->sbuf move)
                for sbl in range(4):
                    sb = 4 * sc + sbl
                    vps = vpsp.tile([P, DL], f32, name=f'v_ps{sb}', tag='vps')
                    for c in range(ND):
                        nc.tensor.matmul(
                            vps[:], r(xt[:, c, ts(sbl, P)]), r(wv_sb[:, c, :]),
                            start=(c == 0), stop=(c == ND - 1))
                    for h in range(HPC):
                        nc.vector.tensor_add(
                            v_sb[:, sb, h * (HD + 1):h * (HD + 1) + HD],
                            vps[:, ts(h, HD)], bvb_sb[:, ts(h, HD)])

            # ---------------- attention + out_proj ----------------
            for c in range(NSC):        # query chunk: s columns [512c, 512c+512)
                for t in range(2):      # head pair; heads 2t (rows 0:64), 2t+1
                    kb_max = min(NB, 4 * c + 6)   # key blocks 0..kb_max-1
                    avo = [avop.tile([HD + 1, 512], f32,
                                     name=f'avo{c}_{2 * t + hi}', tag='avo')
                           for hi in range(2)]
                    for kb in range(kb_max):
                        z = max(0, kb - 4 * c - 2)   # fully-masked sub-blocks
                        n0 = P * z
                        lb = kb - 2 - 4 * c          # banded sub-block index
                        sps = [mmp.tile([P, 512], f32,
                                        name=f's_ps{c}_{t}_{kb}_{hi}', tag='mm')
                               for hi in range(2)]
                        # the two heads' K=64 matmuls sit in distinct PE row
                        # groups (partitions 0:64 / 64:128) and overlap
                        for hi in range(2):
                            nc.tensor.matmul(
                                sps[hi][:, n0:],
                                r(kT_sb[t][64 * hi:64 * hi + 64, ts(kb, P)]),
                                r(qT_sb[t][64 * hi:64 * hi + 64,
                                           512 * c + n0:512 * (c + 1)]),
                                start=True, stop=True)
                        for hi in range(2):
                            h = 2 * t + hi
                            if 0 <= lb < 4:
                                nc.vector.tensor_add(sps[hi][:, ts(lb, P)],
                                                     sps[hi][:, ts(lb, P)],
                                                     mask_sb[:])
                            et = epool.tile([P, 512], bf16,
                                            name=f'exp{c}_{h}_{kb}', tag='exp')
                            nc.scalar.activation(et[:, n0:], sps[hi][:, n0:],
                                                 AF.Exp)
                            nc.tensor.matmul(
                                avo[hi][:, n0:],
                                r(v_sb[:, kb, h * (HD + 1):(h + 1) * (HD + 1)]),
                                r(et[:, n0:]),
                                start=(kb == 0), stop=(kb == kb_max - 1),
                                skip_group_check=True)
                    # normalize: avo rows 0..63 = sum(attn*V), row 64 = sum(exp)
                    for hi in range(2):
                        h = 2 * t + hi
                        rsb = rpool.tile([1, 512], f32, name=f'r{c}_{h}',
                                         tag='r')
                        act_reciprocal(rsb[:], avo[hi][64:65, :])
                        rb = rpool.tile([HD, 512], f32, name=f'rb{c}_{h}',
                                        tag='rb')
                        nc.gpsimd.partition_broadcast(rb[:], rsb[:])
                        nc.vector.tensor_mul(
                            aoT_sb[t][64 * hi:64 * hi + 64, ts(c, 512)],
                            avo[hi][0:HD, :], rb[:])
                # out_proj for s-blocks 4c..4c+3 (need all heads of chunk c)
                for sbl in range(4):
                    m = 4 * c + sbl
                    for n in range(2):
                        ops = vpsp.tile([P, 512], f32, name=f'o_ps{m}_{n}',
                                        tag='vps')
                        for t in range(2):
                            nc.tensor.matmul(ops[:],
                                             r(aoT_sb[t][:, ts(m, P)]),
                                             r(wo_sb[:, t, ts(n, 512)]),
                                             start=(t == 0), stop=(t == 1))
                        ost = opool.tile([P, 512], f32, name=f'ost{m}_{n}',
                                         tag='ost')
                        nc.vector.tensor_copy(ost[:], ops[:])
                        nc.sync.dma_start(out[ts(m, P), ts(n, 512)], ost[:])

    nc.compile()
    return nc


def _get_module():
    if 'nc' not in _MODULE_CACHE:
        _MODULE_CACHE['nc'] = _build_module()
    return _MODULE_CACHE['nc']


def _make_in_maps(x, in_proj_w, in_proj_b, out_proj_w):
    import ml_dtypes
    bf = ml_dtypes.bfloat16
    x = np.asarray(x, np.float32)
    in_proj_w = np.asarray(in_proj_w, np.float32)
    in_proj_b = np.asarray(in_proj_b, np.float32)
    out_proj_w = np.asarray(out_proj_w, np.float32)

    mask = np.where(np.arange(P)[:, None] <= np.arange(P)[None, :],
                    np.float32(0.0), NEG).astype(np.float32)

    xT = [np.ascontiguousarray(x[b].T) for b in range(B)]
    in_maps = []
    for core in range(NCORES):
        b, hg = core // 4, core % 4
        sl = slice(DL * hg, DL * hg + DL)
        wq = in_proj_w[0 * D:1 * D][sl]
        wk = in_proj_w[1 * D:2 * D][sl]
        wv = in_proj_w[2 * D:3 * D][sl]
        bq = in_proj_b[0 * D:1 * D][sl]
        bk = in_proj_b[1 * D:2 * D][sl]
        bv = in_proj_b[2 * D:3 * D][sl]
        in_maps.append({
            'xT': xT[b].astype(bf),
            'wqT': np.ascontiguousarray(wq.T).astype(bf),
            'wkT': np.ascontiguousarray(wk.T).astype(bf),
            'wvT': np.ascontiguousarray(wv.T).astype(bf),
            'woT': np.ascontiguousarray(out_proj_w[:, sl].T).astype(bf),
            'bq2': np.ascontiguousarray((bq * SCALE).reshape(2, P).T),
            'bk2': np.ascontiguousarray(bk.reshape(2, P).T),
            'bvrow': bv.reshape(1, DL).copy(),
            'maskT': mask,
            'onescol': np.ones((P, 16, 1), ml_dtypes.bfloat16),
        })
    return in_maps


def kernel(x, in_proj_w, in_proj_b, out_proj_w, out_proj_b):
    global LAST_RESULTS
    _install_ntff_shim()
    from concourse import bass_utils

    nc = _get_module()
    in_maps = _make_in_maps(x, in_proj_w, in_proj_b, out_proj_w)
    res = bass_utils.run_bass_kernel_spmd(
        nc, in_maps, core_ids=list(range(NCORES)),
        trace=TRACE,
        **({'trace_cores': TRACE_CORES} if TRACE_CORES else {}))
    LAST_RESULTS = res

    out = np.zeros((B, S, D), np.float32)
    for core in range(NCORES):
        out[core // 4] += res.results[core]['out']
    out += np.asarray(out_proj_b, np.float32)
    return out
